# revision 21
# baseline (speedup 1.0000x reference)
"""GATConv x2 + pools on 8 Trainium2 NeuronCores.

Sharding: one graph per core (edges are within-graph by construction:
src and dst share the same graph offset g*N), so no cross-core comms.

Per core, per GAT layer:
  dense phase : psum = h @ [W | W@Msrc | W@Mdst] (f16 matmuls)
                -> xp table rows [xp(260)|a_src(4)] f16 to DRAM (768B pitch,
                %256 for dma_gather); a_dst [128,4] kept in SBUF per tile.
  edge phase  : per 128-node dst tile, ONE bulk dma_gather by src of
                (xp|a_src); a_dst per edge via transposed-one-hot matmul
                OT^T @ adst_tile on the tensor engine (edges of a chunk all
                target this dst tile, so the gather is a 128-row one-hot);
                logits = lrelu(a_src+a_dst); w = exp(logits)
                (segment-max skipped: alpha is exactly invariant to the
                shift, and |logits| <~ 2 so exp is safe);
                msg = xp * w  (w broadcast 65x via an ACT expand-write);
                one-hot scatter-matmul: psum[128n, 260] += O^T @ [msg|w].
  epilogue    : out = psum_msg * recip(psum_denom) + bias;
                h' = elu(out)+1 = max(out,0) + exp(min(out,0))
                (the +1 is corrected in downstream weights host-side);
                pool x = h'.pw + (pb - sum(pw)); layer1 also transposes h'
                into h1T (f16) for layer-2's dense matmul.

The dma_gather is index-rate bound (~7.7ns/index on the gpsimd engine,
independent of row size), so the kernel minimizes gather *count*: one
gather per (tile, layer); everything else rides in the gathered row or
is computed via one-hot matmuls.

Self-contained: hardcodes shapes from the problem spec.
"""

import numpy as np

B, N, F_IN = 8, 4096, 128
E = 524288
H, C = 4, 64
D1 = H * C  # 256
P = 128
NTILES = N // P  # 32
TROW = 384  # f16 table row: [(xp_h+b_h|1)x4 (260) | a_src 4 | pad] = 768B

_CACHE = {}
_SKIP = set()  # debug timing: subset of {"gather","logit","eb","msg","oh","mm","epi"}


def _preprocess_edges(edge_index):
    """Per-core sorted/padded edge structure shared by one traced program."""
    src_all = np.asarray(edge_index[0]).astype(np.int64)
    dst_all = np.asarray(edge_index[1]).astype(np.int64)
    g = dst_all // N
    per_core = []
    counts = np.zeros((B, NTILES), np.int64)
    for b in range(B):
        m = g == b
        loops = np.arange(N, dtype=np.int64)
        s = np.concatenate([src_all[m] - b * N, loops])
        d = np.concatenate([dst_all[m] - b * N, loops])
        order = np.argsort(d, kind="stable")
        s, d = s[order], d[order]
        per_core.append((s, d))
        counts[b] = np.bincount(d // P, minlength=NTILES)
    nchunks = np.maximum(1, -(-counts.max(axis=0) // P)).astype(int)  # per tile
    TC = int(nchunks.sum())
    EPAD = TC * P
    idx_src = np.zeros((B, EPAD), np.int16)
    dstloc = np.full((B, P, TC), 255.0, np.float32)
    dlrow = np.full((B, EPAD), 255.0, np.float16)
    for b, (s, d) in enumerate(per_core):
        pos = 0
        ccur = 0
        off = 0
        for t in range(NTILES):
            cnt = int(counts[b, t])
            nct = int(nchunks[t])
            L = nct * P
            se = np.zeros(L, np.int64)
            dl = np.full(L, 255.0, np.float64)
            se[:cnt] = s[off : off + cnt]
            dl[:cnt] = d[off : off + cnt] - t * P
            off += cnt
            idx_src[b, pos : pos + L] = se
            dstloc[b, :, ccur : ccur + nct] = dl.reshape(nct, P).T
            dlrow[b, pos : pos + L] = dl
            pos += L
            ccur += nct
    # dma_gather wrapped idx layout: [128, EPAD//16]; idx i at [i%16, i//16],
    # replicated across the 8 groups of 16 partitions.
    def wrap(a):
        w = a.reshape(B, EPAD // 16, 16).transpose(0, 2, 1)
        return np.ascontiguousarray(np.tile(w, (1, 8, 1)))

    dlb = np.ascontiguousarray(
        np.broadcast_to(dlrow[:, None, :], (B, P, EPAD)).astype(np.float16)
    )
    return tuple(int(x) for x in nchunks), wrap(idx_src), dstloc, dlb


def _aug_w(W, att_s, att_d):
    """[ (W_h | 0) x4 heads | W@Msrc | W@Mdst ] -> [K, 268].
    The 65th column of each head block becomes a ones column (via the
    bias row), so the scatter rhs gets msg and denom from one scalar-mult."""
    K = W.shape[0]
    out = np.zeros((K, 268), np.float32)
    Msrc = np.zeros((D1, H), np.float32)
    Mdst = np.zeros((D1, H), np.float32)
    for h in range(H):
        out[:, h * 65 : h * 65 + C] = W[:, h * C : (h + 1) * C]
        Msrc[h * C : (h + 1) * C, h] = att_s[h]
        Mdst[h * C : (h + 1) * C, h] = att_d[h]
    out[:, 260:264] = W @ Msrc
    out[:, 264:268] = W @ Mdst
    return out


def _dma_gather_raw(nc, out_ap, in_ap, idxs_ap, num_idxs, elem_size, elem_step):
    """dma_gather with arbitrary elem_size (bytes read per row); the table
    pitch (elem_step) must still be a multiple of 256B. HW-validated."""
    from concourse import mybir as mb
    gp = nc.gpsimd
    dt_size = mb.dt.size(in_ap.dtype)
    stride_bytes = elem_step * dt_size
    assert stride_bytes % 256 == 0
    _in_ap = gp.lower_ap_dma(in_ap, for_custom_bir_dma=True)
    _idxs_ap = gp.lower_ap(idxs_ap)
    _out_ap = gp.lower_ap(out_ap)
    return gp.add_instruction(
        mb.InstDMAGatherAnt(
            name=nc.get_next_instruction_name(),
            ins=[*_in_ap, _idxs_ap, gp.lower_val_access(gp.to_reg(num_idxs))],
            outs=[_out_ap],
            transpose=False,
            num_idxs=num_idxs,
            elem_size=elem_size,
            stride_bytes_256=stride_bytes // 256,
            gen_mode=0,
            single_packet=False,
            queue_num=0,
            sbuf_tokens_per_rank=0,
            sbuf_free_dim_per_rank=0,
            sbuf_free_dim_pad_per_rank=0,
            sbuf_byte_offset=0,
        )
    )


def _build_program(nchunks, num_cores, n_nodes=N, repeat=1):
    import concourse.bass as bass
    import concourse.tile as tile
    from concourse import bacc, mybir
    from concourse.masks import make_identity

    F16, F32, I16, I32 = (
        mybir.dt.float16,
        mybir.dt.float32,
        mybir.dt.int16,
        mybir.dt.int32,
    )
    AF = mybir.ActivationFunctionType
    OP = mybir.AluOpType
    ntiles = n_nodes // P
    TC = int(sum(nchunks))
    EPAD = TC * P

    nc = bacc.Bacc(
        "TRN2", target_bir_lowering=False, debug=False, num_devices=num_cores
    )
    xT_d = nc.declare_dram_parameter("xT", [P, n_nodes], F16, isOutput=False)
    w1_d = nc.declare_dram_parameter("W1a", [F_IN, 268], F16, isOutput=False)
    w2_d = nc.declare_dram_parameter("W2a", [D1 + 1, 268], F16, isOutput=False)
    bp_d = nc.declare_dram_parameter("bp", [4, D1], F32, isOutput=False)
    pbe_d = nc.declare_dram_parameter("pbe", [1, 2], F32, isOutput=False)
    bc1_d = nc.declare_dram_parameter("bc1", [1, 268], F16, isOutput=False)
    isrc_d = nc.declare_dram_parameter("isrc", [P, EPAD // 16], I16, isOutput=False)
    dl_d = nc.declare_dram_parameter("dstloc", [P, TC], F32, isOutput=False)
    dlb_d = nc.declare_dram_parameter("dlb", [P, EPAD], F16, isOutput=False)
    out_d = nc.declare_dram_parameter("out", [3, n_nodes], F32, isOutput=True)
    table_d = [
        nc.dram_tensor("table1", [n_nodes, TROW], F16),
        nc.dram_tensor("table2", [n_nodes, TROW], F16),
    ]

    from contextlib import ExitStack

    with tile.TileContext(nc) as tc, ExitStack() as ctx:
        pp = ctx.enter_context(tc.tile_pool(name="persist", bufs=1))
        gpool = ctx.enter_context(tc.tile_pool(name="gather", bufs=3))
        dpool = ctx.enter_context(tc.tile_pool(name="dlbuf", bufs=3))
        epool = ctx.enter_context(tc.tile_pool(name="ebuf", bufs=3))
        lpool = ctx.enter_context(tc.tile_pool(name="logits", bufs=3))
        rpool = ctx.enter_context(tc.tile_pool(name="rhs", bufs=12))
        opool = ctx.enter_context(tc.tile_pool(name="onehot", bufs=3))
        hpool = ctx.enter_context(tc.tile_pool(name="hwork", bufs=2))
        spool = ctx.enter_context(tc.tile_pool(name="small", bufs=2))
        stpool = ctx.enter_context(tc.tile_pool(name="stage", bufs=3))
        pacc = ctx.enter_context(tc.tile_pool(name="pacc", bufs=3, space="PSUM"))
        pdense = ctx.enter_context(tc.tile_pool(name="pdense", bufs=2, space="PSUM"))
        ptrans = ctx.enter_context(tc.tile_pool(name="ptrans", bufs=2, space="PSUM"))
        pmisc = ctx.enter_context(tc.tile_pool(name="pmisc", bufs=1, space="PSUM"))

        # ---- persistent loads & constants ----
        xT_sb = pp.tile([P, n_nodes], F16, tag="xT")
        for q in range(4):
            nc.sync.dma_start(
                xT_sb[:, q * (n_nodes // 4) : (q + 1) * (n_nodes // 4)],
                xT_d[:, q * (n_nodes // 4) : (q + 1) * (n_nodes // 4)],
            )
        w1_sb = pp.tile([F_IN, 268], F16, tag="w1")
        nc.sync.dma_start(w1_sb[:], w1_d[:])
        w2a_sb = pp.tile([P, 268], F16, tag="w2a")
        nc.sync.dma_start(w2a_sb[:], w2_d[0:P, :])
        w2b_sb = pp.tile([P, 268], F16, tag="w2b")
        nc.sync.dma_start(w2b_sb[:], w2_d[P : 2 * P, :])
        w2c_sb = pp.tile([1, 268], F16, tag="w2c")
        nc.sync.dma_start(w2c_sb[:], w2_d[2 * P : 2 * P + 1, :])
        bp_rows = []
        for r in range(4):
            rt = pp.tile([1, D1], F32, tag=f"bprow{r}")
            nc.sync.dma_start(rt[:], bp_d[r : r + 1, :])
            bp_rows.append(rt)
        pbe_sb = pp.tile([1, 2], F32, tag="pbe")
        nc.sync.dma_start(pbe_sb[:], pbe_d[:])
        bc1e_sb = pp.tile([1, 268], F16, tag="bc1e")
        nc.sync.dma_start(bc1e_sb[:], bc1_d[:])
        isrc_sb = pp.tile([P, EPAD // 16], I16, tag="isrc")
        nc.sync.dma_start(isrc_sb[:], isrc_d[:])
        dl_sb = pp.tile([P, TC], F32, tag="dstloc")
        nc.sync.dma_start(dl_sb[:], dl_d[:])

        iota_i = pp.tile([P, P], I32, tag="iotai")
        nc.gpsimd.iota(iota_i[:], pattern=[[1, P]], base=0, channel_multiplier=0)
        iota16 = pp.tile([P, P], F16, tag="iota16")
        nc.vector.tensor_copy(iota16[:], iota_i[:])
        iotac_i = pp.tile([P, 1], I32, tag="iotaci")
        nc.gpsimd.iota(iotac_i[:], pattern=[[1, 1]], base=0, channel_multiplier=1)
        iotac32 = pp.tile([P, 1], F32, tag="iotac32")
        nc.vector.tensor_copy(iotac32[:], iotac_i[:])
        ident = pp.tile([P, P], F32, tag="ident")
        make_identity(nc, ident[:])
        ones1 = pp.tile([1, P], F32, tag="ones1")
        nc.vector.memset(ones1[:], 1.0)
        ones1h = pp.tile([1, P], F16, tag="ones1h")
        nc.vector.memset(ones1h[:], 1.0)
        inv128 = pp.tile([P, 1], F16, tag="inv128")
        nc.vector.memset(inv128[:], 1.0 / F_IN)

        # broadcast constants [128, 256] via ones-matmul
        def bcast_row(row_ap, width, tag):
            ps = pmisc.tile([P, width], F32, space="PSUM", tag="pmisc")
            nc.tensor.matmul(ps[:], lhsT=ones1[:], rhs=row_ap, start=True, stop=True)
            t = pp.tile([P, width], F32, tag=tag)
            nc.vector.tensor_copy(t[:], ps[:])
            return t

        pw1_bc = bcast_row(bp_rows[2][:], D1, "pw1bc")
        pw2_bc = bcast_row(bp_rows[3][:], D1, "pw2bc")
        pbe_bc = bcast_row(pbe_sb[0:1, :], 2, "pbebc")

        h1T_sb = pp.tile([P, 2 * n_nodes], F16, tag="h1T")
        x1_sb = pp.tile([P, ntiles], F32, tag="x1")
        x2_sb = pp.tile([P, ntiles], F32, tag="x2")
        x0_sb = pp.tile([1, n_nodes], F32, tag="x0")
        # per-layer per-tile a_dst rows, kept on-chip: [128, ntiles*4] f16
        adst1_sb = pp.tile([P, ntiles * 4], F16, tag="adst1")
        adst2_sb = pp.tile([P, ntiles * 4], F16, tag="adst2")
        adst_sb = [adst1_sb, adst2_sb]

        # ---- dense phase: write [xp|a_src] table rows, keep a_dst in SBUF ----
        def dense(layer, t):
            ps = pdense.tile([P, 268], F32, space="PSUM", tag="pdense")
            if layer == 0:
                nc.tensor.matmul(
                    ps[:], lhsT=xT_sb[:, t * P : (t + 1) * P], rhs=w1_sb[:],
                    start=True, stop=False,
                )
                nc.tensor.matmul(
                    ps[:], lhsT=ones1h[:], rhs=bc1e_sb[:], start=False, stop=True,
                )
            else:
                nc.tensor.matmul(
                    ps[:], lhsT=h1T_sb[:, t * P : t * P + P], rhs=w2a_sb[:],
                    start=True, stop=False,
                )
                nc.tensor.matmul(
                    ps[:], lhsT=h1T_sb[:, n_nodes + t * P : n_nodes + t * P + P],
                    rhs=w2b_sb[:], start=False, stop=False,
                )
                nc.tensor.matmul(
                    ps[:], lhsT=ones1h[:], rhs=w2c_sb[:], start=False, stop=True,
                )
            stg = stpool.tile([P, 264], F16, tag="stg")
            if layer == 0:
                nc.vector.tensor_copy(stg[:], ps[:, 0:264])
                nc.vector.tensor_copy(
                    adst_sb[layer][:, t * 4 : (t + 1) * 4], ps[:, 264:268]
                )
            else:
                nc.scalar.copy(stg[:], ps[:, 0:264])
                nc.scalar.copy(adst_sb[layer][:, t * 4 : (t + 1) * 4], ps[:, 264:268])
            nc.sync.dma_start(table_d[layer][t * P : (t + 1) * P, 0:264], stg[:])

        # ---- edge phase for one node tile ----
        # front: gather + one-hot builds + a_dst OT-matmuls + logits + exp +
        # per-chunk msg mults.  The scatter matmuls are issued one tile LATER
        # (edge_scatter) so the PE runs tile t+1's OT-matmuls before tile t's
        # scatter chain — this breaks the loop-carried
        # mults->scatter->OT->logits->exp->mults dependency that otherwise
        # paces the gather stream.
        def edge_front(layer, t, c0, nct):
            L = nct * P
            gb = gpool.tile([P, nct, 264], F16, tag="gb")
            if "gather" not in _SKIP:
                _dma_gather_raw(
                    nc, gb[:], table_d[layer][:, 0:264],
                    isrc_sb[:, c0 * 8 : c0 * 8 + L // 16],
                    L, 264, TROW,
                )
            else:
                nc.vector.memset(gb[:], 1.0)
            # a_dst per edge: transposed one-hot matmul against this tile's
            # adst rows.  OT[d, e] = (dlb[e] == d); psum_a[e, :] = adst[dst[e]].
            dlb_t = dpool.tile([P, L], F16, tag="dlb")
            nc.sync.dma_start(dlb_t[:], dlb_d[:, c0 * P : c0 * P + L])
            ot = opool.tile([P, L], F16, tag="ot")
            nc.vector.tensor_scalar(
                ot[:], dlb_t[:], iotac32[:], None, OP.is_equal
            )
            # batched one-hot build for the scatter matmuls
            oh_all = opool.tile([P, nct, P], F16, tag="oh")
            if "oh" not in _SKIP:
                nc.vector.tensor_tensor(
                    oh_all[:],
                    dl_sb[:, c0 : c0 + nct].unsqueeze(2).to_broadcast([P, nct, P]),
                    iota16[:].unsqueeze(1).to_broadcast([P, nct, P]),
                    op=OP.is_equal,
                )
            else:
                nc.vector.memset(oh_all[:], 0.0)
            ps_all = pacc.tile([P, 260 + nct * 4], F32, space="PSUM", tag="pacc")
            ps_a = ps_all[:, 260 : 260 + nct * 4]
            for j in range(nct):
                nc.tensor.matmul(
                    ps_a[:, j * 4 : (j + 1) * 4],
                    lhsT=ot[:, j * P : (j + 1) * P],
                    rhs=adst_sb[layer][:, t * 4 : (t + 1) * 4],
                    start=True, stop=True,
                )
            lg2 = lpool.tile([P, nct, 4], F32, tag="lg2")
            if "logit" in _SKIP:
                nc.vector.memset(lg2[:], 0.0)
            if "logit" not in _SKIP:
                lg = lpool.tile([P, nct, 4], F32, tag="lg")
                nc.vector.tensor_tensor(
                    lg[:], gb[:, :, 260:264],
                    ps_a.rearrange("p (c a) -> p c a", a=4), op=OP.add,
                )
                nc.vector.scalar_tensor_tensor(
                    lg2[:], lg[:], 0.2, lg[:], op0=OP.mult, op1=OP.max
                )
            eb = epool.tile([P, nct, 260], F16, tag="eb")
            if "eb" not in _SKIP:
                nc.scalar.activation(
                    eb[:].rearrange("p c (a b) -> p c a b", a=H),
                    lg2[:].unsqueeze(3).to_broadcast([P, nct, 4, 65]),
                    AF.Exp,
                )
            else:
                nc.vector.memset(eb[:], 1.0)
            rhs_groups = []
            for j0 in range(0, nct, 4):
                rhs4 = rpool.tile([P, 4, 260], F16, tag="rhs")
                jn = min(4, nct - j0)
                if "msg" not in _SKIP:
                    nc.vector.tensor_tensor(
                        rhs4[:, 0:jn, :], gb[:, j0 : j0 + jn, 0:260],
                        eb[:, j0 : j0 + jn, :], op=OP.mult,
                    )
                else:
                    nc.vector.memset(rhs4[:], 1.0)
                rhs_groups.append(rhs4)
            return (layer, t, nct, ps_all, oh_all, rhs_groups)

        def edge_scatter(state):
            layer, t, nct, ps_all, oh_all, rhs_groups = state
            ps_acc = ps_all[:, 0:260]
            for j in range(nct):
                if "mm" not in _SKIP:
                    nc.tensor.matmul(
                        ps_acc, lhsT=oh_all[:, j, :],
                        rhs=rhs_groups[j // 4][:, j % 4, :],
                        start=(j == 0), stop=(j == nct - 1),
                    )

        def edge_epi_a(state):
            """First epilogue half: frees the PSUM bank and issues the exp on
            ACT one iteration before edge_epi_b consumes it (so the exp is
            queued ahead of the next tile's eb on the in-order ACT engine)."""
            layer, t, nct, ps_all, oh_all, rhs_groups = state
            ps_acc = ps_all[:, 0:260]
            if "epi" in _SKIP:
                return None
            if "mm" in _SKIP:
                nc.vector.memset(ps_acc, 1.0)
            rec = spool.tile([P, 4], F32, tag="rec")
            nc.vector.reciprocal(
                rec[:], ps_acc.rearrange("p (a b) -> p a b", a=H)[:, :, C]
            )
            y = hpool.tile([P, D1], F32, tag="y")
            nc.vector.tensor_tensor(
                y[:].rearrange("p (a b) -> p a b", a=H),
                ps_acc.rearrange("p (a b) -> p a b", a=H)[:, :, 0:C],
                rec[:].unsqueeze(2).to_broadcast([P, 4, C]),
                op=OP.mult,
            )
            t1 = hpool.tile([P, D1], F32, tag="t1")
            nc.vector.tensor_scalar(t1[:], y[:], 0.0, None, OP.min)
            t2 = hpool.tile([P, D1], F32, tag="t2")
            nc.scalar.activation(t2[:], t1[:], AF.Exp)
            return (layer, t, y, t2)

        def edge_epi_b(stateb, pw_bc, xcol):
            if stateb is None:
                return
            layer, t, y, t2 = stateb
            hp = hpool.tile([P, D1], F32, tag="hp")
            nc.vector.scalar_tensor_tensor(
                hp[:], y[:], 0.0, t2[:], op0=OP.max, op1=OP.add
            )
            scr = hpool.tile([P, D1], F32, tag="scr")
            nc.vector.scalar_tensor_tensor(
                scr[:], hp[:], 1.0, pw_bc[:], op0=OP.mult, op1=OP.mult,
                accum_out=xcol,
            )
            if layer == 0:
                for fh in range(2):
                    pst = ptrans.tile([P, P], F32, space="PSUM", tag="ptrans")
                    nc.tensor.transpose(
                        pst[:], hp[:, fh * P : (fh + 1) * P], ident[:]
                    )
                    nc.vector.tensor_copy(
                        h1T_sb[:, fh * n_nodes + t * P : fh * n_nodes + t * P + P],
                        pst[:],
                    )

        def assemble_x(x_sb, pbe_col, row):
            xa = spool.tile([P, ntiles], F32, tag="xa")
            nc.vector.tensor_scalar(
                xa[:], x_sb[:], pbe_bc[:, pbe_col : pbe_col + 1], None, OP.add
            )
            pst = ptrans.tile([ntiles, P], F32, space="PSUM", tag="ptrans")
            nc.tensor.transpose(pst[:], xa[:], ident[:])
            xo = spool.tile([ntiles, P], F32, tag="xo")
            nc.vector.tensor_copy(xo[:], pst[:])
            nc.sync.dma_start(
                out_d[row : row + 1, :].rearrange("a (b c) -> (a b) c", b=ntiles),
                xo[:],
            )

        for _rep in range(repeat):
            # ---- layer 1 dense ----
            for t in range(ntiles):
                dense(0, t)
            # ---- x0 = mean_f x (PE idle while layer-1 gathers start) ----
            for k in range(n_nodes // 512):
                ps = pmisc.tile([1, 512], F32, space="PSUM", tag="pmisc")
                nc.tensor.matmul(
                    ps[:], lhsT=inv128[:], rhs=xT_sb[:, k * 512 : (k + 1) * 512],
                    start=True, stop=True,
                )
                nc.scalar.copy(x0_sb[:, k * 512 : (k + 1) * 512], ps[:])
            nc.sync.dma_start(out_d[0:1, :], x0_sb[:])
            # ---- layer 1 edges: three-stage software pipeline ----
            # iteration t issues front(t), scatter(t-1), epiA(t-1),
            # epiB(t-2)+dense(t-2)
            c0 = 0
            p1 = None
            a1 = a2 = None
            for t in range(ntiles):
                cur = edge_front(0, t, c0, nchunks[t])
                na = None
                if p1 is not None:
                    edge_scatter(p1)
                    na = edge_epi_a(p1)
                if a2 is not None:
                    edge_epi_b(a2, pw1_bc, x1_sb[:, a2[1] : a2[1] + 1])
                    dense(1, a2[1])
                a2, a1 = a1, na
                p1 = cur
                c0 += nchunks[t]
            edge_scatter(p1)
            na = edge_epi_a(p1)
            for a in (a2, a1, na):
                if a is not None:
                    edge_epi_b(a, pw1_bc, x1_sb[:, a[1] : a[1] + 1])
                    dense(1, a[1])
            assemble_x(x1_sb, 0, 1)
            # ---- layer 2 edges ----
            c0 = 0
            p1 = None
            a1 = a2 = None
            for t in range(ntiles):
                cur = edge_front(1, t, c0, nchunks[t])
                na = None
                if p1 is not None:
                    edge_scatter(p1)
                    na = edge_epi_a(p1)
                if a2 is not None:
                    edge_epi_b(a2, pw2_bc, x2_sb[:, a2[1] : a2[1] + 1])
                a2, a1 = a1, na
                p1 = cur
                c0 += nchunks[t]
            edge_scatter(p1)
            na = edge_epi_a(p1)
            for a in (a2, a1, na):
                if a is not None:
                    edge_epi_b(a, pw2_bc, x2_sb[:, a[1] : a[1] + 1])
            assemble_x(x2_sb, 1, 2)

    nc.compile()
    return nc


def _prepare_inputs(x, edge_index, W1, att_src1, att_dst1, b1, W2, att_src2,
                    att_dst2, b2, pw1, pb1, pw2, pb2):
    nchunks, isrc_w, dstloc, dlb = _preprocess_edges(edge_index)
    W1a = _aug_w(np.asarray(W1, np.float32), np.asarray(att_src1, np.float32),
                 np.asarray(att_dst1, np.float32))
    W2a = _aug_w(np.asarray(W2, np.float32), np.asarray(att_src2, np.float32),
                 np.asarray(att_dst2, np.float32))
    W2corr = -W2a.sum(axis=0, keepdims=True)
    b2a = np.asarray(b2, np.float32)
    for h in range(H):
        W2corr[0, h * 65 : h * 65 + C] += b2a[h * C : (h + 1) * C]
        W2corr[0, h * 65 + C] = 1.0  # ones column
    W2aug = np.concatenate([W2a, W2corr], axis=0).astype(np.float16)
    pw1 = np.asarray(pw1, np.float32)
    pw2 = np.asarray(pw2, np.float32)
    bp = np.stack([
        np.asarray(b1, np.float32), np.asarray(b2, np.float32),
        pw1[:, 0], pw2[:, 0],
    ]).astype(np.float32)
    pbe = np.array(
        [[float(pb1[0]) - float(pw1.sum()), float(pb2[0]) - float(pw2.sum())]],
        np.float32,
    )
    bc1 = np.zeros((1, 268), np.float32)
    b1a = np.asarray(b1, np.float32)
    for h in range(H):
        bc1[0, h * 65 : h * 65 + C] = b1a[h * C : (h + 1) * C]
        bc1[0, h * 65 + C] = 1.0  # ones column
    x = np.asarray(x, np.float32)
    in_maps = []
    for b in range(B):
        in_maps.append({
            "xT": np.ascontiguousarray(x[b].T).astype(np.float16),
            "W1a": W1a.astype(np.float16),
            "W2a": W2aug,
            "bp": bp,
            "pbe": pbe,
            "bc1": bc1.astype(np.float16),
            "isrc": isrc_w[b],
            "dstloc": np.ascontiguousarray(dstloc[b]),
            "dlb": dlb[b],
        })
    return nchunks, in_maps


_RUN_KWARGS = {}
_LAST_RESULT = None


def kernel(**inputs):
    global _LAST_RESULT
    from concourse.bass_utils import run_bass_kernel_spmd

    nchunks, in_maps = _prepare_inputs(**inputs)
    key = nchunks
    if key not in _CACHE:
        _CACHE[key] = _build_program(nchunks, B)
    nc = _CACHE[key]
    res = run_bass_kernel_spmd(nc, in_maps, list(range(B)), **_RUN_KWARGS)
    _LAST_RESULT = res
    out = np.stack([res.results[b]["out"].reshape(3 * N) for b in range(B)])
    return out.astype(np.float32)
